# revision 1
# baseline (speedup 1.0000x reference)
"""Bass/Trainium2 kernel for nn_DiagonalTraining (per-anti-diagonal Linear).

Math: for each anti-diagonal i of x[B,S,S] (entries x[b,r,i-r], r<=i),
apply Linear_i (weights W[i,:i+1,:i+1], bias b[i,:i+1]) to the gathered
vector and scatter back reversed. Equivalent to:
    D[b,i,j] = x[b,j,i-j] (j<=i else 0)
    out[b,i,k] = sum_j W[i,k,j] * D[b,i,j] + b[i,k]
    new_x[b,r,c] = out[b,r+c,c] if r+c < S else x[b,r,c]

Device does the einsum (memory-bound: streams the valid triangle of W);
gather/scatter/bias are tiny O(S^2) host ops.

Sharding: interleaved over diagonals — core c owns i = c, c+8, ..., c+504
(slot m holds diagonal 8m+c, k-padded to L=8(m+1)). All cores run one
identical SPMD program; padding rows/cols of W and D are zero by
construction so results are exact.

Performance architecture (measured on HW):
- Everything fp8e4: W scaled by 32 on host (scale-invariant quantization,
  rel err 9.2e-3 from W alone), D bf16, and the PSUM result out*32 fits
  fp8e4 output staging directly (total rel err 1.30e-2 < 2e-2).
- The PE with 4-way tile_position col-packing streams ~2.5 cols/cycle
  (~770GB/s of fp8 W) — 1.8x the ~420GB/s DMA rate, so the kernel is
  stream-paced end to end. Only stream continuity, startup latency and
  the post-last-byte tail matter.
- W image is packed per-superchunk as [full-height chunk block][partial
  last-chunk block at the group-shared height H]: PE streaming time is
  per COLUMN (independent of partition height), so trimming the mostly
  -zero last j-chunks saves ~13% of W traffic for free. Matmuls slice
  both operands to the transferred height (never read unwritten SBUF).
- Each superchunk is fetched as big DMAs split across the sync+scalar
  queues (multi-KB per-partition descriptors, both queues saturated).
  Small groups stream first (instant PE start from resident wsmall),
  the tiny g4 is streamed last so the post-stream tail is short.
- Mid-loop output DMAs go on gpsimd only (a compute-gated descriptor on
  a W queue would stall all later W transfers behind it); the last two
  processed groups' outputs use the by-then-idle sync/scalar queues.
"""

import sys

sys.path.insert(0, "/opt/trn_rl_repo")

import numpy as np

B = 8
S = 512
NCORES = 8
M = 64  # diagonal slots per core
LBAR = [8 * (m + 1) for m in range(M)]  # k-padded diagonal length per slot
NQ = [1 if m < 16 else (m // 16 + 1) for m in range(M)]  # j-chunks per slot
QOFF = np.cumsum([0] + NQ).tolist()  # chunk index offset per slot in dt image
DTOT = QOFF[M]  # 160 chunks
# wsmall region (slots 0..15): column offsets within [0, SMALL_TOT)
WCUM = np.cumsum([0] + [NQ[m] * LBAR[m] for m in range(M)]).tolist()
SMALL_TOT = WCUM[16]  # 1088
G = 16  # groups of 4 slots sharing a PSUM bank
LG = [32 * (g + 1) for g in range(G)]  # group output width
OCUM = np.cumsum([0] + LG).tolist()
OTOT = OCUM[G]  # 4352
WSCALE = 32.0  # fp8 W scale; PSUM holds out*32 which fits fp8e4 directly

# Stream order of the W superchunks (all groups in a chunk share nq).
# Small resident groups 0-3 are computed first; g4 streams LAST so the
# post-stream tail (PE+copy+out of the final group) is minimal.
SUPER = [[5, 6, 7], [8, 9], [10, 11], [12], [13], [14], [15], [4]]
PROC_ORDER = [3, 2, 1, 0] + [g for ch in SUPER for g in ch]


# True height of group g's partial (last) j-chunk: 32(g+1) - 128(nq-1).
HTRUE = {g: 32 * (g + 1) - 128 * (NQ[4 * g] - 1) for g in range(4, 16)}
# Toggle for the stacked partial-chunk trim (-0.63MB of W traffic).
TRIM_STACK = False
# Stacked partial regions: pack two groups' partial chunks vertically into
# one full-height DMA (plain [H, cols] DMAs only engage H/8 of the 16 SDMA
# engines — measured slower despite fewer bytes). The offset member's
# matmul uses tile_position=(poff, 32t) with both operands partition-
# sliced at poff (PE row-group tiling: h<=32 at 96, h<=64 at 64).
# Members: (group, row_offset). g7/g11/g15 partials are 128-high already
# and stay inline in their superchunks; g5's 64-high region rides alone.
STACKS = [
    [(6, 0), (4, 96)],
    [(10, 0), (8, 96)],
    [(14, 0), (12, 96)],
    [(13, 0), (9, 64)],
    [(5, 0)],
]
# Fetch each stack together with the superchunk of its earliest-processed
# member.
STACK_WITH = {0: (5, 6, 7), 1: (8, 9), 3: (8, 9), 2: (12,), 4: (5, 6, 7)}


def _build_wlayout():
    """Column layout of the m>=16 W image: per superchunk the full-height
    chunks (plus inline 128-high partials for g7/g11/g15), then the
    stacked partial regions."""
    scol = {}
    info = {}
    col = SMALL_TOT
    for ch in SUPER:
        nq = NQ[4 * ch[0]]
        f0 = col
        for g in ch:
            for t in range(4):
                m = 4 * g + t
                for q in range(nq - 1):
                    scol[(m, q)] = col
                    col += LBAR[m]
        for g in ch:
            if HTRUE[g] == 128 or not TRIM_STACK:
                for t in range(4):
                    m = 4 * g + t
                    scol[(m, nq - 1)] = col
                    col += LBAR[m]
        info[tuple(ch)] = (f0, col)
    stacks = []
    poffm = {}
    for members in STACKS if TRIM_STACK else []:
        s0 = col
        wid = 0
        hmax = 0
        for g, poff in members:
            c = s0
            for t in range(4):
                m = 4 * g + t
                scol[(m, NQ[m] - 1)] = c
                poffm[m] = poff
                c += LBAR[m]
            wid = max(wid, c - s0)
            hmax = max(hmax, poff + HTRUE[g])
        stacks.append((s0, s0 + wid, hmax))
        col = s0 + wid
    return scol, info, stacks, poffm, col


SCOL, CHINFO, STACKINFO, POFFM, WTOT2 = _build_wlayout()

MODE = "fp8"
_compiled = {}


def build_program(mode=MODE):
    """Build the SPMD Bass program (same instructions on all 8 cores)."""
    import concourse.mybir as mybir
    import concourse.tile as tile
    from concourse import bacc

    assert mode == "fp8"
    wdt = mybir.dt.float8e4
    ddt = mybir.dt.bfloat16
    odt = mybir.dt.float8e4
    f32 = mybir.dt.float32

    nc = bacc.Bacc("TRN2")
    wimg = nc.dram_tensor("wimg", [128, WTOT2], wdt, kind="ExternalInput")
    dt_in = nc.dram_tensor("dt", [128, DTOT * B], ddt, kind="ExternalInput")
    out = nc.dram_tensor("out", [128, OTOT], odt, kind="ExternalOutput")

    with tile.TileContext(nc) as tc:
        with (
            tc.tile_pool(name="dpool", bufs=1) as dpool,
            tc.tile_pool(name="wspool", bufs=1) as wspool,
            tc.tile_pool(name="wpool", bufs=8) as wpool,
            tc.tile_pool(name="stpool", bufs=5) as stpool,
            tc.tile_pool(name="opool", bufs=16) as opool,
            tc.tile_pool(name="psum", bufs=8, space="PSUM") as psum_pool,
        ):
            # Each hw-DGE queue loses ~0.7us of engine time per DMA
            # (descriptor writes + semaphore handshake), so W is striped
            # across THREE queues (sync/scalar/gpsimd) in whole-chunk DMAs
            # — one queue's dead time hides under the others' transfers.
            dtall = dpool.tile([128, DTOT * B], ddt)
            dsplit = QOFF[16] * B
            dmid = dsplit + (DTOT * B - dsplit) // 2
            nc.sync.dma_start(dtall[:, 0:dsplit], dt_in[:, 0:dsplit])
            wsmall = wspool.tile([128, SMALL_TOT], wdt)
            nc.scalar.dma_start(wsmall[:], wimg[:, 0:SMALL_TOT])
            nc.sync.dma_start(dtall[:, dsplit:dmid], dt_in[:, dsplit:dmid])
            nc.scalar.dma_start(dtall[:, dmid:], dt_in[:, dmid:])

            def fetch_chunk(ch):
                """Fetch one superchunk: whole range split in halves across
                the sync+scalar queues (the empirically fastest pattern:
                multi-KB per-partition descriptors, both queues loaded
                with the same chunk so it lands in bytes/420GB/s)."""
                f0, c1 = CHINFO[tuple(ch)]
                wtile = wpool.tile([128, 8544], wdt, tag="w")
                cm = f0 + (c1 - f0) // 2
                nc.sync.dma_start(wtile[0:128, 0 : cm - f0], wimg[:, f0:cm])
                nc.scalar.dma_start(
                    wtile[0:128, cm - f0 : c1 - f0], wimg[:, cm:c1]
                )
                return wtile

            stiles = {}

            def fetch_stack(sid, eng):
                s0, s1, hmax = STACKINFO[sid]
                st = stpool.tile([128, 1872], wdt, tag="st")
                eng.dma_start(st[0:hmax, 0 : s1 - s0], wimg[0:hmax, s0:s1])
                return st

            STACK_OF = {}
            if TRIM_STACK:
                for sid_, members_ in enumerate(STACKS):
                    for g_, _ in members_:
                        STACK_OF[g_] = sid_
            CHUNK_OF = {g: tuple(ch) for ch in SUPER for g in ch}
            fetched = {}
            n_sdma = 0
            for g in PROC_ORDER:
                ps = psum_pool.tile([128, 512], f32, tag="ps")
                if g >= 4:
                    ch = CHUNK_OF[g]
                    if ch not in fetched:
                        fetched[ch] = fetch_chunk(ch)
                        for sid_, wch in STACK_WITH.items() if TRIM_STACK else []:
                            if wch == ch:
                                eng = nc.sync if n_sdma % 2 else nc.scalar
                                n_sdma += 1
                                stiles[sid_] = fetch_stack(sid_, eng)
                    wtile = fetched[ch]
                    f0 = CHINFO[ch][0]
                for t in range(4):
                    m = 4 * g + t
                    L = LBAR[m]
                    nq = NQ[m]
                    for q in range(nq):
                        poff = 0
                        if m < 16:
                            h = 128
                            rhs = wsmall[0:128, WCUM[m] : WCUM[m] + L]
                        elif q == nq - 1 and g in STACK_OF:
                            sid = STACK_OF[g]
                            st = stiles[sid]
                            s0 = STACKINFO[sid][0]
                            poff = POFFM[m]
                            h = HTRUE[g]
                            c0 = SCOL[(m, q)] - s0
                            rhs = st[poff : poff + h, c0 : c0 + L]
                        else:
                            h = 128
                            c0 = SCOL[(m, q)] - f0
                            rhs = wtile[0:128, c0 : c0 + L]
                        nc.tensor.matmul(
                            ps[32 * t : 32 * t + B, 0:L],
                            lhsT=dtall[
                                poff : poff + h,
                                (QOFF[m] + q) * B : (QOFF[m] + q + 1) * B,
                            ],
                            rhs=rhs,
                            start=(q == 0),
                            stop=(q == nq - 1),
                            tile_position=(poff, 32 * t),
                        )
                ot = opool.tile([128, 512], odt, tag="ostage")
                if g % 2 == 1:
                    nc.vector.tensor_copy(ot[0:128, 0 : LG[g]], ps[0:128, 0 : LG[g]])
                else:
                    nc.scalar.copy(ot[0:128, 0 : LG[g]], ps[0:128, 0 : LG[g]])
                # Mid-loop out DMAs must not share the W queues (in-order
                # queues: a compute-gated descriptor would stall later W
                # transfers). The last two processed groups are emitted
                # after all W fetches, so sync/scalar are safe and faster
                # than the tail of gpsimd's issue chain.
                oeng = {15: nc.sync, 4: nc.scalar}.get(g, nc.gpsimd)
                oeng.dma_start(out[:, OCUM[g] : OCUM[g + 1]], ot[0:128, 0 : LG[g]])

    nc.compile()
    return nc


def _get_program(mode=MODE):
    if mode not in _compiled:
        _compiled[mode] = build_program(mode)
    return _compiled[mode]


def _prep_inputs(x, W, mode=MODE):
    """Host-side shard prep: gather diagonals of x, pack W SBUF images."""
    import ml_dtypes

    wnp = np.dtype(ml_dtypes.float8_e4m3)
    dnp = np.dtype(ml_dtypes.bfloat16)
    wscale = np.float32(WSCALE)

    i_idx = np.arange(S)[:, None]
    r_idx = np.arange(S)[None, :]
    cols = (i_idx - r_idx) % S
    valid = (r_idx <= i_idx)[None]
    D = np.where(valid, x[:, r_idx, cols], np.float32(0.0))  # [B, S(i), S(j)]

    stacked_groups = (
        {g for st in STACKS for g, _ in st} if TRIM_STACK else set()
    )

    def _chunk_place(m, q):
        """(partition offset, height limit) for chunk q of slot m. Stacked
        last chunks are clipped to their true height so the 0-offset
        member's zero rows never clobber its stack partner's rows."""
        g = m // 4
        if m >= 16 and q == NQ[m] - 1 and g in stacked_groups:
            return POFFM[m], HTRUE[g]
        return 0, 128

    in_maps = []
    for c in range(NCORES):
        Wc = W[c::8]  # [M, S(k), S(j)]
        WIMG = np.zeros((128, WTOT2), dtype=wnp)
        for m in range(M):
            L, nq = LBAR[m], NQ[m]
            for q in range(nq):
                # img[j, k] = Wc[m, k, 128q + j] * wscale
                poff, hlim = _chunk_place(m, q)
                blk = Wc[m, 0:L, 128 * q : 128 * (q + 1)] * wscale  # [k, j]
                img = blk.T.astype(wnp, copy=False)  # [j<=128, k=L]
                jh = min(img.shape[0], hlim, 128 - poff)
                c0 = WCUM[m] + q * L if m < 16 else SCOL[(m, q)]
                WIMG[poff : poff + jh, c0 : c0 + L] = img[0:jh]
        # DT[j, qoff_m + q, b] = D[b, 8m+c, 128q+j]; last chunks of
        # stacked groups live at the matching partition offset.
        Dc = D[:, c::8, :]  # [B, M, S]
        DT = np.zeros((128, DTOT * B), dtype=dnp)
        for m in range(M):
            nq = NQ[m]
            for q in range(nq):
                poff, hlim = _chunk_place(m, q)
                arr = Dc[:, m, 128 * q : 128 * (q + 1)].T  # [j<=128, B]
                jh = min(arr.shape[0], hlim, 128 - poff)
                DT[poff : poff + jh, (QOFF[m] + q) * B : (QOFF[m] + q + 1) * B] = (
                    arr[0:jh].astype(dnp, copy=False)
                )
        in_maps.append({"wimg": WIMG, "dt": DT})
    return in_maps


def _postprocess(x, bvec, results, mode=MODE):
    """Assemble per-core outputs, undo W scale, add bias, scatter back."""
    inv_scale = np.float32(1.0 / WSCALE)
    out_full = np.empty((B, S, S), dtype=np.float32)
    for c in range(NCORES):
        o = np.asarray(results[c]["out"]).astype(np.float32)  # [128, OTOT]
        for g in range(G):
            blk = o[:, OCUM[g] : OCUM[g + 1]].reshape(4, 32, LG[g])[:, 0:B]
            for t in range(4):
                m = 4 * g + t
                out_full[:, 8 * m + c, 0 : LBAR[m]] = blk[t, :, 0 : LBAR[m]]
    out_full *= inv_scale
    out_full += bvec[None]
    rr = np.arange(S)[:, None]
    cc = np.arange(S)[None, :]
    diag = rr + cc
    new_x = np.where(
        (diag < S)[None], out_full[:, np.minimum(diag, S - 1), cc], x
    ).astype(np.float32)
    return new_x


def kernel_run(x, W, b, mode=MODE, trace=False):
    from concourse.bass_utils import run_bass_kernel_spmd

    nc = _get_program(mode)
    in_maps = _prep_inputs(x, W, mode)
    res = run_bass_kernel_spmd(nc, in_maps, list(range(NCORES)), trace=trace)
    return _postprocess(x, b, res.results, mode), res


def kernel(x, W, b):
    out, _ = kernel_run(np.asarray(x), np.asarray(W), np.asarray(b))
    return out

